# revision 38
# baseline (speedup 1.0000x reference)
"""Trainium2 Bass kernel for nn_AttentionSubsample (8-core SPMD).

Sharding: batch N=2 x 4 head-groups (3 heads each) -> 8 cores, no
collectives.  Each core computes q/k/v projections for its head group
(K/V on the stride-2 subsampled positions only), per-head attention with
softmax folded as exp -> denominator via an appended ones-column in V ->
divide, and its partial output projection in transposed layout.  The
host sums the 4 per-batch partials (bf16) in fp32 and adds the bias.

Schedule: x^T loads in three q-column blocks so the first projection
chains start ~30% into the load; the pre-beat prefix computes q(c0), all
of k (m0 chunks + fused q/k-m1 chains), and the row-swapped q/k
duplicates (per-piece DMAs right behind the input in the sync-queue
FIFO).  The scalar-engine exp stream then starts at ~30us and stays
>95% dense: remaining q-chunks and all v chains fill chunk 0's beats,
and each later chunk's beats interleave the previous chunk's PV /
divide / out-proj as coarse fillers (whole matmul chains keep the PE
warm; fine-grained splitting re-throttles the HAM clock gate).

PE work savings vs the naive form: q-m1 and k-m1 share one fused chain
(k computed densely, even columns copied out); the out-proj h2 (K=64)
matmuls run as concurrent pairs on opposite PE row halves (duplicated
wp-h2 rows + duplicated attn-h2); score matmuls alternate PE row halves
by k-tile parity via the qT/kT duplicates so consecutive scores overlap
on the array.  bf16 outputs are DMA'd in two-chunk batches (1.75KB
partition lines); the last chunk flushes per-piece during the drain.

Layout notes:
 - The spatial stride-2 subsample of K/V equals taking even rows of the
   flattened [3136, 768] batch (196 is even), i.e. even columns of x^T.
 - All device matmuls run in bf16 (fp32 PSUM accumulation).
"""

import sys

for _p in ("/opt/trn_rl_repo",):
    if _p not in sys.path:
        sys.path.insert(0, _p)

import numpy as np
import ml_dtypes

import concourse.bass as bass  # noqa: F401  (registers engines)
import concourse.tile as tile
from concourse import bacc, mybir
from concourse.bass_utils import run_bass_kernel_spmd

BFNP = ml_dtypes.bfloat16
F32 = mybir.dt.float32
F32R = mybir.dt.float32r
BF16 = mybir.dt.bfloat16
AF = mybir.ActivationFunctionType

N, T, S, D = 2, 16, 196, 768
H, HD = 12, 64
Q = T * S              # 3136 query positions per batch
KP = T * (S // 2)      # 1568 subsampled key positions
HPG = 3                # heads per group (12 heads / 4 groups)
GD = HPG * HD          # 192 channels per head group
SC = (D // H) ** -0.5  # 0.125 attention scale
CH = 448               # q-chunk size (3136 = 7 * 448)
NCH = Q // CH          # 7
NKT = 13               # k tiles: 12 * 128 + 32
KTL = 32               # last k-tile height
NDK = D // 128         # 6 contraction tiles for the projections
N_CORES = 8

# exp groups over k-tiles: PSUM scores tile holds 3 banks (512-aligned)
EXP_GROUPS = [(0, 1), (2, 3), (4, 5), (6, 7), (8, 9), (10, 11), (12,)]

TRACE = False          # test.py flips this for profiled runs
LAST_RESULTS = {}      # exec_time_ns etc. stashed here on traced runs

_CACHE = {}


def _ksize(kt):
    return 128 if kt < NKT - 1 else KTL


def _head_pos(h):
    """(block, partition base) of head h inside the 2-block qT/kT tiles."""
    return (0, 0) if h == 0 else ((0, 64) if h == 1 else (1, 0))


def _build_nc():
    nc = bacc.Bacc(
        "TRN2", target_bir_lowering=False, debug=False, num_devices=N_CORES
    )
    xT = nc.dram_tensor("xT", [D, Q], BF16, kind="ExternalInput").ap()
    wq = nc.dram_tensor("wq", [D, GD], BF16, kind="ExternalInput").ap()
    wk = nc.dram_tensor("wk", [D, GD], BF16, kind="ExternalInput").ap()
    wv = nc.dram_tensor("wv", [D, GD], BF16, kind="ExternalInput").ap()
    wp = nc.dram_tensor("wp", [GD, D], BF16, kind="ExternalInput").ap()
    out = nc.dram_tensor("out", [D, Q], BF16, kind="ExternalOutput").ap()

    with tile.TileContext(nc) as tc:
        _body(tc, xT, wq, wk, wv, wp, out)
    nc.compile()
    return nc


def _body(tc, xT, wq, wk, wv, wp, out):
    nc = tc.nc
    with (
        tc.tile_pool(name="persist", bufs=1) as P,
        tc.tile_pool(name="es", bufs=6) as ES,
        tc.tile_pool(name="inv", bufs=2) as INV,
        tc.tile_pool(name="ot", bufs=1) as OT,
        tc.tile_pool(name="scps", bufs=3, space="PSUM") as SCPS,
        tc.tile_pool(name="numps", bufs=2, space="PSUM") as NUMPS,
    ):
        # ---- persistent SBUF tensors -------------------------------------
        # m0 slices of wq/wk (heads h0+h1), per contraction ktile
        wq_sb = P.tile([128, NDK * 128], BF16, tag="wq")
        nc.sync.dma_start(
            wq_sb[:].rearrange("p (a g) -> p a g", a=NDK),
            wq.rearrange("(a p) g -> p a g", p=128)[:, :, 0:128],
        )
        wk_sb = P.tile([128, NDK * 128], BF16, tag="wk")
        nc.sync.dma_start(
            wk_sb[:].rearrange("p (a g) -> p a g", a=NDK),
            wk.rearrange("(a p) g -> p a g", p=128)[:, :, 0:128],
        )
        # fused m1 weights: cols 0:64 = wq[:,128:192], 64:128 = wk[:,128:192]
        wqk1 = P.tile([128, NDK * 128], BF16, tag="wqk1")
        nc.sync.dma_start(
            wqk1[:].rearrange("p (a g) -> p a g", a=NDK)[:, :, 0:64],
            wq.rearrange("(a p) g -> p a g", p=128)[:, :, 128:192],
        )
        nc.sync.dma_start(
            wqk1[:].rearrange("p (a g) -> p a g", a=NDK)[:, :, 64:128],
            wk.rearrange("(a p) g -> p a g", p=128)[:, :, 128:192],
        )
        # x^T in per-ktile tiles, loaded in three q-column blocks so the
        # first projection chains start after ~30% of the load
        xt = [P.tile([128, Q], BF16, tag=f"xt{kt}", name=f"xt{kt}")
              for kt in range(NDK)]
        for b0, b1 in ((0, 896), (896, 1792), (1792, Q)):
            for kt in range(NDK):
                nc.sync.dma_start(
                    xt[kt][:, b0:b1],
                    xT[kt * 128 : (kt + 1) * 128, b0:b1],
                )
        wv_sb = P.tile([128, NDK * GD], BF16, tag="wv")
        nc.sync.dma_start(
            wv_sb[:].rearrange("p (a g) -> p a g", a=NDK),
            wv.rearrange("(a p) g -> p a g", p=128),
        )
        # wp: h0 rows at partitions 0:64 and h1 at 64:128 (K=128 matmul)
        wp_sb = P.tile([128, D], BF16, tag="wp")
        nc.sync.dma_start(wp_sb[0:HD, 0:D], wp[0:HD, :])
        nc.sync.dma_start(wp_sb[HD:128, 0:D], wp[HD : 2 * HD, :])
        # h2 rows duplicated in both halves for the concurrent K=64 pairs
        wph2 = P.tile([128, D], BF16, tag="wph2")
        nc.sync.dma_start(wph2[0:HD, 0:D], wp[2 * HD : 3 * HD, :])
        nc.sync.dma_start(wph2[HD:128, 0:D], wp[2 * HD : 3 * HD, :])
        ones_f = P.tile([128, HD], F32, tag="ones_f")
        nc.vector.memset(ones_f[:], 1.0)
        ones = P.tile([128, HD], F32R, tag="ones")
        nc.vector.tensor_copy(ones[:], ones_f[:])

        # PE warm-up: dummy matmuls with no DMA dependence run during the
        # input-load window, so the HAM clock gate is already at 8/8 when
        # the first projection chains issue.
        wu = P.tile([128, 512], BF16, tag="wu")
        nc.vector.memset(wu[:], 1.0)
        wups = NUMPS.tile([128, 512], F32, tag="num", name="wups")
        for _ in range(12):
            nc.tensor.matmul(
                wups[0:128, 0:512], wu[:, 0:128], wu[:, 0:512],
                start=True, stop=True,
            )

        qT = P.tile([128, 2 * Q], BF16, tag="qT")       # q^T: rows=[h0|h1], [h2]
        kT = P.tile([128, 2 * KP], BF16, tag="kT")      # k^T subsampled
        qT_dup = P.tile([128, 2 * Q], BF16, tag="qTd")  # row-halves swapped
        kT_dup = P.tile([128, 2 * KP], BF16, tag="kTd")
        vv = P.tile([128, HPG * NKT * 65], BF16, tag="v")  # v + ones col, [k, 65]/tile
        # attn out: block 0 rows 0:64 = h0, rows 64:128 = h1; block 1 = h2
        attn = P.tile([128, 2 * Q], BF16, tag="attn")

        # ones columns of the v tiles
        nc.vector.memset(vv[:, 64 : HPG * NKT * 65 : 65], 1.0)

        # ---- A: projection chains, emitted in x-block readiness order ----
        _apool = [(SCPS, "sc", 1024), (SCPS, "sc", 1024), (SCPS, "sc", 1024),
                  (NUMPS, "num", 512), (NUMPS, "num", 512)]
        _ai = [0]
        _inbeat = [False]

        def _aps():
            if _inbeat[0]:
                pool, tag, w = NUMPS, "num", 512
            else:
                pool, tag, w = _apool[_ai[0] % 5]
            _ai[0] += 1
            return pool.tile([128, w], F32, tag=tag, name=f"aps{_ai[0]}")

        def q_m0(c):
            ps = _aps()
            for kt in range(NDK):
                nc.tensor.matmul(
                    ps[0:128, 0:CH],
                    wq_sb[:, kt * 128 : (kt + 1) * 128],
                    xt[kt][:, c * CH : (c + 1) * CH],
                    start=(kt == 0),
                    stop=(kt == NDK - 1),
                )
            nc.vector.tensor_copy(
                qT[0:128, c * CH : (c + 1) * CH], ps[0:128, 0:CH]
            )
            nc.sync.dma_start(
                qT_dup[HD:128, c * CH : (c + 1) * CH],
                qT[0:HD, c * CH : (c + 1) * CH],
            )
            nc.sync.dma_start(
                qT_dup[0:HD, c * CH : (c + 1) * CH],
                qT[HD:128, c * CH : (c + 1) * CH],
            )

        def k_m0(c0, csz):
            ps = _aps()
            for kt in range(NDK):
                nc.tensor.matmul(
                    ps[0:128, 0:csz],
                    wk_sb[:, kt * 128 : (kt + 1) * 128],
                    xt[kt][:, 2 * c0 : 2 * (c0 + csz) : 2],
                    start=(kt == 0),
                    stop=(kt == NDK - 1),
                )
            nc.vector.tensor_copy(kT[0:128, c0 : c0 + csz], ps[0:128, 0:csz])
            nc.sync.dma_start(
                kT_dup[HD:128, c0 : c0 + csz], kT[0:HD, c0 : c0 + csz]
            )
            nc.sync.dma_start(
                kT_dup[0:HD, c0 : c0 + csz], kT[HD:128, c0 : c0 + csz]
            )

        def qk_m1(c):
            # fused: rows 0:64 = q_h2 (all positions), rows 64:128 = k_h2
            # (dense; even columns are the subsampled keys)
            ps = _aps()
            for kt in range(NDK):
                nc.tensor.matmul(
                    ps[0:128, 0:CH],
                    wqk1[:, kt * 128 : (kt + 1) * 128],
                    xt[kt][:, c * CH : (c + 1) * CH],
                    start=(kt == 0),
                    stop=(kt == NDK - 1),
                )
            nc.vector.tensor_copy(
                qT[0:HD, Q + c * CH : Q + (c + 1) * CH], ps[0:HD, 0:CH]
            )
            nc.vector.tensor_copy(
                kT[0:HD, KP + c * 224 : KP + (c + 1) * 224],
                ps[HD:128, 0:CH:2],
            )
            nc.sync.dma_start(
                qT_dup[HD:128, Q + c * CH : Q + (c + 1) * CH],
                qT[0:HD, Q + c * CH : Q + (c + 1) * CH],
            )
            nc.sync.dma_start(
                kT_dup[HD:128, KP + c * 224 : KP + (c + 1) * 224],
                kT[0:HD, KP + c * 224 : KP + (c + 1) * 224],
            )

        def v_kt(kt_m):
            msz = _ksize(kt_m)
            ps = _aps()
            for kt in range(NDK):
                base = 2 * (kt_m * 128)
                nc.tensor.matmul(
                    ps[0:msz, 0:GD],
                    xt[kt][:, base : base + 2 * msz : 2],
                    wv_sb[:, kt * GD : (kt + 1) * GD],
                    start=(kt == 0),
                    stop=(kt == NDK - 1),
                )
            for h in range(HPG):
                slot = (h * NKT + kt_m) * 65
                nc.vector.tensor_copy(
                    vv[0:msz, slot : slot + 64],
                    ps[0:msz, h * HD : (h + 1) * HD],
                )

        kchunks = [(0, 448), (448, 448), (896, 448), (1344, 224)]
        # pre-beat prefix in x-block readiness order: everything chunk-0's
        # (and h2's) score groups need; q(1..6) and v chains run as fillers
        q_m0(0); k_m0(*kchunks[0]); qk_m1(0); qk_m1(1)
        k_m0(*kchunks[1]); qk_m1(2); qk_m1(3)
        k_m0(*kchunks[2]); k_m0(*kchunks[3])
        _inbeat[0] = True

        # ---- B/C: attention, software-pipelined by one q-chunk -----------
        def emit_score_group(h, c, es, grp):
            blk, pb = _head_pos(h)
            if True:
                ng = len(grp)
                scp = SCPS.tile([128, 1024], F32, tag="sc")
                pmax = _ksize(grp[-1])
                for j, kt in enumerate(grp):
                    ksz = _ksize(kt)
                    if kt % 2 == 0:
                        sk, sq, base = kT, qT, pb
                    else:
                        sk, sq, base = kT_dup, qT_dup, HD - pb
                    nc.tensor.matmul(
                        scp[0:ksz, j * 512 : j * 512 + CH],
                        sk[base : base + HD, blk * KP + kt * 128 : blk * KP + kt * 128 + ksz],
                        sq[base : base + HD, blk * Q + c * CH : blk * Q + (c + 1) * CH],
                        start=True,
                        stop=True,
                        tile_position=(base, 0),
                    )
                src = scp[0:pmax, 0 : ng * 512].rearrange(
                    "p (a b) -> p a b", b=512
                )[:, :, 0:CH] if ng > 1 else scp[0:pmax, 0:CH]
                dst = es[
                    0:pmax, grp[0] * CH : (grp[-1] + 1) * CH
                ].rearrange("p (a b) -> p a b", b=CH) if ng > 1 else es[
                    0:pmax, grp[0] * CH : grp[0] * CH + CH
                ]
                nc.scalar.activation(dst, src, AF.Exp, scale=SC)

        def pv_part(h, c, es):
            num = NUMPS.tile([128, 512], F32, tag="num")
            for kt in range(NKT):
                ksz = _ksize(kt)
                slot = (h * NKT + kt) * 65
                nc.tensor.matmul(
                    num[0:65, 0:CH],
                    vv[0:ksz, slot : slot + 65],
                    es[0:ksz, kt * CH : (kt + 1) * CH],
                    start=(kt == 0),
                    stop=(kt == NKT - 1),
                )
            den = INV.tile([128, CH], F32, tag="den")
            nc.vector.tensor_copy(den[0:1, :], num[64:65, 0:CH])
            nsb = INV.tile([128, CH], F32, tag="nsb", bufs=3)
            nc.vector.tensor_copy(nsb[0:64, :], num[0:64, 0:CH])
            inv = INV.tile([128, CH], F32, tag="inv")
            nc.vector.reciprocal_approx_fast(inv[0:1, :], den[0:1, :])
            invr = INV.tile([128, CH], F32R, tag="invr", bufs=3)
            nc.vector.tensor_copy(invr[0:1, :], inv[0:1, :])
            return num, nsb, invr

        def rep_mult(h, c, num, nsb, invr):
            # broadcast 1/den across rows 0:64 of the drained num tile
            # (PV rows already staged into nsb; den row copied out)
            nc.tensor.matmul(
                num[0:64, 0:CH], ones[0:1, 0:64], invr[0:1, 0:CH],
                start=True, stop=True,
            )
            if h == 0:
                dst = attn[0:64, c * CH : (c + 1) * CH]
            elif h == 1:
                dst = attn[64:128, c * CH : (c + 1) * CH]
            else:
                dst = attn[0:64, Q + c * CH : Q + (c + 1) * CH]
            nc.vector.tensor_tensor(
                dst, nsb[0:64, :], num[0:64, 0:CH], op=mybir.AluOpType.mult
            )
            if h == 2:
                # duplicate h2 into rows 64:128 for the concurrent proj pair
                nc.sync.dma_start(
                    attn[64:128, Q + c * CH : Q + (c + 1) * CH],
                    attn[0:64, Q + c * CH : Q + (c + 1) * CH],
                )

        _otst = {}

        def proj_pair(c, mp):
            # out-proj for m-blocks (2*mp, 2*mp+1): two K=128 h0h1 matmuls
            # plus an h2 K=64 pair on opposite PE row halves; bf16 outputs
            # are batched per m-block over chunks (0,1), (2,3), (4,5,6)
            me, mo = 2 * mp, 2 * mp + 1
            pp = SCPS.tile([128, 1024], F32, tag="sc", name=f"pj{mp}")
            nc.tensor.matmul(
                pp[0:128, 0:CH],
                wp_sb[0:128, me * 128 : (me + 1) * 128],
                attn[0:128, c * CH : (c + 1) * CH],
                start=True,
                stop=False,
            )
            nc.tensor.matmul(
                pp[0:128, 512 : 512 + CH],
                wp_sb[0:128, mo * 128 : (mo + 1) * 128],
                attn[0:128, c * CH : (c + 1) * CH],
                start=True,
                stop=False,
            )
            nc.tensor.matmul(
                pp[0:128, 0:CH],
                wph2[0:HD, me * 128 : (me + 1) * 128],
                attn[0:HD, Q + c * CH : Q + (c + 1) * CH],
                start=False,
                stop=True,
                tile_position=(0, 0),
            )
            nc.tensor.matmul(
                pp[0:128, 512 : 512 + CH],
                wph2[HD:128, mo * 128 : (mo + 1) * 128],
                attn[HD:128, Q + c * CH : Q + (c + 1) * CH],
                start=False,
                stop=True,
                tile_position=(HD, 0),
            )
            cb = 2 * (c // 2)                    # output batch start chunk
            half_c = c - cb
            for m, bank in ((me, 0), (mo, 1)):
                if half_c == 0:
                    _otst[m] = OT.tile([128, 2 * CH], BF16, tag=f"otm{m}",
                                       name=f"otm{m}")
                ot = _otst[m]
                nc.vector.tensor_copy(
                    ot[:, half_c * CH : (half_c + 1) * CH],
                    pp[0:128, bank * 512 : bank * 512 + CH],
                )
                if half_c == 1 or c == NCH - 1:
                    w = (half_c + 1) * CH
                    nc.sync.dma_start(
                        out[m * 128 : (m + 1) * 128, cb * CH : cb * CH + w],
                        ot[:, 0:w],
                    )

        pv_queue = []
        for c in range(NCH):
            es = [ES.tile([128, NKT * CH], BF16, tag="es", name=f"es{c}_{h}")
                  for h in range(HPG)]
            fillers = []
            if c == 0:
                # qk1(4..6) first: h2's score groups need them by slot ~7
                fillers = [(lambda cc: lambda: qk_m1(cc))(cc)
                           for cc in (4, 5, 6)]
                fillers += [(lambda cc: lambda: q_m0(cc))(cc)
                            for cc in range(1, NCH)]
                fillers += [(lambda k: lambda: v_kt(k))(kt_m)
                            for kt_m in range(NKT)]
                # interleave: qm0 chains first (they gate later chunks)
                fillers = fillers[:6] + fillers[6:]
            if pv_queue:
                pc = pv_queue[0][1]
                state = {}

                def mk_pv(h, ppc, pes):
                    def f():
                        state[h] = pv_part(h, ppc, pes)
                    return f

                def mk_rep(h, ppc):
                    def f():
                        num, nsb, invr = state[h]
                        rep_mult(h, ppc, num, nsb, invr)
                    return f

                def mk_proj(mp, ppc):
                    def f():
                        proj_pair(ppc, mp)
                    return f

                f_pv = [mk_pv(h, ppc, pes) for h, ppc, pes in pv_queue]
                f_rep = [mk_rep(h, ppc) for h, ppc, pes in pv_queue]
                f_pj = [mk_proj(mp, pc) for mp in range(3)]
                fillers = [
                    f_pv[0], f_pv[1], f_pv[2],
                    f_rep[0], f_rep[1], f_rep[2],
                    f_pj[0], f_pj[1], f_pj[2],
                ]
            # 2-ktile groups each hold one complete row-pair, so fillers
            # can slot between any groups without breaking pairing
            fi = 0
            gi = 0
            for h in range(HPG):
                for grp in EXP_GROUPS:
                    emit_score_group(h, c, es[h], grp)
                    gi += 1
                    if gi % 2 == 0 and fi < len(fillers):
                        fillers[fi]()
                        fi += 1
            while fi < len(fillers):
                fillers[fi]()
                fi += 1
            pv_queue = [(0, c, es[0]), (1, c, es[1]), (2, c, es[2])]
        parts = [pv_part(h, ppc, pes) for h, ppc, pes in pv_queue]
        for (h, ppc, pes), (num, nsb, invr) in zip(pv_queue, parts):
            rep_mult(h, ppc, num, nsb, invr)
        for mp in range(3):
            proj_pair(NCH - 1, mp)


def _get_nc():
    if "nc" not in _CACHE:
        _CACHE["nc"] = _build_nc()
    return _CACHE["nc"]


def kernel(x, W_qkv, W_proj, b_proj):
    nc = _get_nc()
    xTs = [
        np.ascontiguousarray(
            x[n].reshape(Q, D).astype(BFNP).T
        )
        for n in range(N)
    ]
    wqs, wks, wvs, wps = [], [], [], []
    for g in range(4):
        c0 = g * GD
        wqs.append(np.ascontiguousarray(W_qkv[:, c0 : c0 + GD].astype(BFNP)))
        wks.append(np.ascontiguousarray(W_qkv[:, D + c0 : D + c0 + GD].astype(BFNP)))
        wvs.append(
            np.ascontiguousarray(W_qkv[:, 2 * D + c0 : 2 * D + c0 + GD].astype(BFNP))
        )
        wps.append(np.ascontiguousarray(W_proj[c0 : c0 + GD, :].astype(BFNP)))
    in_maps = [
        {"xT": xTs[c // 4], "wq": wqs[c % 4], "wk": wks[c % 4],
         "wv": wvs[c % 4], "wp": wps[c % 4]}
        for c in range(N_CORES)
    ]
    res = run_bass_kernel_spmd(nc, in_maps, list(range(N_CORES)), trace=TRACE)
    if TRACE:
        LAST_RESULTS["exec_time_ns"] = res.exec_time_ns
        LAST_RESULTS["mean_exec_time_ns"] = res.mean_exec_time_ns
    out = np.empty((N, T, S, D), np.float32)
    for n in range(N):
        acc = res.results[4 * n]["out"].astype(np.float32)
        for g in range(1, 4):
            acc = acc + res.results[4 * n + g]["out"].astype(np.float32)
        out[n] = (acc.T + b_proj).reshape(T, S, D)
    return out



# revision 39
# speedup vs baseline: 1.0186x; 1.0186x over previous
"""Trainium2 Bass kernel for nn_AttentionSubsample (8-core SPMD).

Sharding: batch N=2 x 4 head-groups (3 heads each) -> 8 cores, no
collectives.  Each core computes q/k/v projections for its head group
(K/V on the stride-2 subsampled positions only), per-head attention with
softmax folded as exp -> denominator via an appended ones-column in V ->
divide, and its partial output projection in transposed layout.  The
host sums the 4 per-batch partials and adds the bias.

Layout notes:
 - The spatial stride-2 subsample of K/V equals taking even rows of the
   flattened [3136, 768] batch (196 is even), i.e. even columns of x^T.
 - All device matmuls run in bf16 (fp32 PSUM accumulation).
"""

import sys

for _p in ("/opt/trn_rl_repo",):
    if _p not in sys.path:
        sys.path.insert(0, _p)

import numpy as np
import ml_dtypes

import concourse.bass as bass  # noqa: F401  (registers engines)
import concourse.tile as tile
from concourse import bacc, mybir
from concourse.bass_utils import run_bass_kernel_spmd

BFNP = ml_dtypes.bfloat16
F32 = mybir.dt.float32
F32R = mybir.dt.float32r
BF16 = mybir.dt.bfloat16
AF = mybir.ActivationFunctionType

N, T, S, D = 2, 16, 196, 768
H, HD = 12, 64
Q = T * S              # 3136 query positions per batch
KP = T * (S // 2)      # 1568 subsampled key positions
HPG = 3                # heads per group (12 heads / 4 groups)
GD = HPG * HD          # 192 channels per head group
SC = (D // H) ** -0.5  # 0.125 attention scale
CH = 448               # q-chunk size (3136 = 7 * 448)
NCH = Q // CH          # 7
NKT = 13               # k tiles: 12 * 128 + 32
KTL = 32               # last k-tile height
NDK = D // 128         # 6 contraction tiles for the projections
N_CORES = 8

# exp groups over k-tiles: PSUM scores tile holds 3 banks (512-aligned)
EXP_GROUPS = [(0, 1), (2, 3), (4, 5), (6, 7), (8, 9), (10, 11), (12,)]

TRACE = False          # test.py flips this for profiled runs
LAST_RESULTS = {}      # exec_time_ns etc. stashed here on traced runs

_CACHE = {}


def _ksize(kt):
    return 128 if kt < NKT - 1 else KTL


def _head_pos(h):
    """(block, partition base) of head h inside the 2-block qT/kT tiles."""
    return (0, 0) if h == 0 else ((0, 64) if h == 1 else (1, 0))


def _build_nc():
    nc = bacc.Bacc(
        "TRN2", target_bir_lowering=False, debug=False, num_devices=N_CORES
    )
    xT = nc.dram_tensor("xT", [D, Q], BF16, kind="ExternalInput").ap()
    wq = nc.dram_tensor("wq", [D, GD], BF16, kind="ExternalInput").ap()
    wk = nc.dram_tensor("wk", [D, GD], BF16, kind="ExternalInput").ap()
    wv = nc.dram_tensor("wv", [D, GD], BF16, kind="ExternalInput").ap()
    wp = nc.dram_tensor("wp", [GD, D], BF16, kind="ExternalInput").ap()
    out = nc.dram_tensor("out", [D, Q], BF16, kind="ExternalOutput").ap()

    with tile.TileContext(nc) as tc:
        _body(tc, xT, wq, wk, wv, wp, out)
    nc.compile()
    return nc


def _body(tc, xT, wq, wk, wv, wp, out):
    nc = tc.nc
    with (
        tc.tile_pool(name="persist", bufs=1) as P,
        tc.tile_pool(name="es", bufs=6) as ES,
        tc.tile_pool(name="inv", bufs=2) as INV,
        tc.tile_pool(name="ot", bufs=1) as OT,
        tc.tile_pool(name="scps", bufs=3, space="PSUM") as SCPS,
        tc.tile_pool(name="numps", bufs=2, space="PSUM") as NUMPS,
    ):
        # ---- persistent SBUF tensors -------------------------------------
        # m0 slices of wq/wk (heads h0+h1), per contraction ktile
        wq_sb = P.tile([128, NDK * 128], BF16, tag="wq")
        nc.sync.dma_start(
            wq_sb[:].rearrange("p (a g) -> p a g", a=NDK),
            wq.rearrange("(a p) g -> p a g", p=128)[:, :, 0:128],
        )
        wk_sb = P.tile([128, NDK * 128], BF16, tag="wk")
        nc.sync.dma_start(
            wk_sb[:].rearrange("p (a g) -> p a g", a=NDK),
            wk.rearrange("(a p) g -> p a g", p=128)[:, :, 0:128],
        )
        # fused m1 weights: cols 0:64 = wq[:,128:192], 64:128 = wk[:,128:192]
        wqk1 = P.tile([128, NDK * 128], BF16, tag="wqk1")
        nc.sync.dma_start(
            wqk1[:].rearrange("p (a g) -> p a g", a=NDK)[:, :, 0:64],
            wq.rearrange("(a p) g -> p a g", p=128)[:, :, 128:192],
        )
        nc.sync.dma_start(
            wqk1[:].rearrange("p (a g) -> p a g", a=NDK)[:, :, 64:128],
            wk.rearrange("(a p) g -> p a g", p=128)[:, :, 128:192],
        )
        # x^T in per-ktile tiles, loaded in three q-column blocks so the
        # first projection chains start after ~30% of the load
        xt = [P.tile([128, Q], BF16, tag=f"xt{kt}", name=f"xt{kt}")
              for kt in range(NDK)]
        for b0, b1 in ((0, 896), (896, 1792), (1792, Q)):
            for kt in range(NDK):
                nc.sync.dma_start(
                    xt[kt][:, b0:b1],
                    xT[kt * 128 : (kt + 1) * 128, b0:b1],
                )
        wv_sb = P.tile([128, NDK * GD], BF16, tag="wv")
        nc.sync.dma_start(
            wv_sb[:].rearrange("p (a g) -> p a g", a=NDK),
            wv.rearrange("(a p) g -> p a g", p=128),
        )
        # wp: h0 rows at partitions 0:64 and h1 at 64:128 (K=128 matmul)
        wp_sb = P.tile([128, D], BF16, tag="wp")
        nc.sync.dma_start(wp_sb[0:HD, 0:D], wp[0:HD, :])
        nc.sync.dma_start(wp_sb[HD:128, 0:D], wp[HD : 2 * HD, :])
        # h2 rows duplicated in both halves for the concurrent K=64 pairs
        wph2 = P.tile([128, D], BF16, tag="wph2")
        nc.sync.dma_start(wph2[0:HD, 0:D], wp[2 * HD : 3 * HD, :])
        nc.sync.dma_start(wph2[HD:128, 0:D], wp[2 * HD : 3 * HD, :])
        ones_f = P.tile([128, HD], F32, tag="ones_f")
        nc.vector.memset(ones_f[:], 1.0)
        ones = P.tile([128, HD], F32R, tag="ones")
        nc.vector.tensor_copy(ones[:], ones_f[:])

        # PE warm-up: dummy matmuls with no DMA dependence run during the
        # input-load window, so the HAM clock gate is already at 8/8 when
        # the first projection chains issue.
        wu = P.tile([128, 512], BF16, tag="wu")
        nc.vector.memset(wu[:], 1.0)
        wups = NUMPS.tile([128, 512], F32, tag="num", name="wups")
        for _ in range(12):
            nc.tensor.matmul(
                wups[0:128, 0:512], wu[:, 0:128], wu[:, 0:512],
                start=True, stop=True,
            )

        qT = P.tile([128, 2 * Q], BF16, tag="qT")       # q^T: rows=[h0|h1], [h2]
        kT = P.tile([128, 2 * KP], BF16, tag="kT")      # k^T subsampled
        qT_dup = P.tile([128, 2 * Q], BF16, tag="qTd")  # row-halves swapped
        kT_dup = P.tile([128, 2 * KP], BF16, tag="kTd")
        vv = P.tile([128, HPG * NKT * 65], BF16, tag="v")  # v + ones col, [k, 65]/tile
        # attn out: block 0 rows 0:64 = h0, rows 64:128 = h1; block 1 = h2
        attn = P.tile([128, 2 * Q], BF16, tag="attn")

        # ones columns of the v tiles
        nc.vector.memset(vv[:, 64 : HPG * NKT * 65 : 65], 1.0)

        # ---- A: projection chains, emitted in x-block readiness order ----
        _apool = [(SCPS, "sc", 1024), (SCPS, "sc", 1024), (SCPS, "sc", 1024),
                  (NUMPS, "num", 512), (NUMPS, "num", 512)]
        _ai = [0]
        _inbeat = [False]

        def _aps():
            if _inbeat[0]:
                pool, tag, w = NUMPS, "num", 512
            else:
                pool, tag, w = _apool[_ai[0] % 5]
            _ai[0] += 1
            return pool.tile([128, w], F32, tag=tag, name=f"aps{_ai[0]}")

        def q_m0(c):
            ps = _aps()
            for kt in range(NDK):
                nc.tensor.matmul(
                    ps[0:128, 0:CH],
                    wq_sb[:, kt * 128 : (kt + 1) * 128],
                    xt[kt][:, c * CH : (c + 1) * CH],
                    start=(kt == 0),
                    stop=(kt == NDK - 1),
                )
            nc.vector.tensor_copy(
                qT[0:128, c * CH : (c + 1) * CH], ps[0:128, 0:CH]
            )
            nc.sync.dma_start(
                qT_dup[HD:128, c * CH : (c + 1) * CH],
                qT[0:HD, c * CH : (c + 1) * CH],
            )
            nc.sync.dma_start(
                qT_dup[0:HD, c * CH : (c + 1) * CH],
                qT[HD:128, c * CH : (c + 1) * CH],
            )

        def k_m0(c0, csz):
            ps = _aps()
            for kt in range(NDK):
                nc.tensor.matmul(
                    ps[0:128, 0:csz],
                    wk_sb[:, kt * 128 : (kt + 1) * 128],
                    xt[kt][:, 2 * c0 : 2 * (c0 + csz) : 2],
                    start=(kt == 0),
                    stop=(kt == NDK - 1),
                )
            nc.vector.tensor_copy(kT[0:128, c0 : c0 + csz], ps[0:128, 0:csz])
            nc.sync.dma_start(
                kT_dup[HD:128, c0 : c0 + csz], kT[0:HD, c0 : c0 + csz]
            )
            nc.sync.dma_start(
                kT_dup[0:HD, c0 : c0 + csz], kT[HD:128, c0 : c0 + csz]
            )

        def qk_m1(c):
            # fused: rows 0:64 = q_h2 (all positions), rows 64:128 = k_h2
            # (dense; even columns are the subsampled keys)
            ps = _aps()
            for kt in range(NDK):
                nc.tensor.matmul(
                    ps[0:128, 0:CH],
                    wqk1[:, kt * 128 : (kt + 1) * 128],
                    xt[kt][:, c * CH : (c + 1) * CH],
                    start=(kt == 0),
                    stop=(kt == NDK - 1),
                )
            nc.vector.tensor_copy(
                qT[0:HD, Q + c * CH : Q + (c + 1) * CH], ps[0:HD, 0:CH]
            )
            nc.vector.tensor_copy(
                kT[0:HD, KP + c * 224 : KP + (c + 1) * 224],
                ps[HD:128, 0:CH:2],
            )
            nc.sync.dma_start(
                qT_dup[HD:128, Q + c * CH : Q + (c + 1) * CH],
                qT[0:HD, Q + c * CH : Q + (c + 1) * CH],
            )
            nc.sync.dma_start(
                kT_dup[HD:128, KP + c * 224 : KP + (c + 1) * 224],
                kT[0:HD, KP + c * 224 : KP + (c + 1) * 224],
            )

        def v_kt(kt_m):
            msz = _ksize(kt_m)
            ps = _aps()
            for kt in range(NDK):
                base = 2 * (kt_m * 128)
                nc.tensor.matmul(
                    ps[0:msz, 0:GD],
                    xt[kt][:, base : base + 2 * msz : 2],
                    wv_sb[:, kt * GD : (kt + 1) * GD],
                    start=(kt == 0),
                    stop=(kt == NDK - 1),
                )
            for h in range(HPG):
                slot = (h * NKT + kt_m) * 65
                nc.vector.tensor_copy(
                    vv[0:msz, slot : slot + 64],
                    ps[0:msz, h * HD : (h + 1) * HD],
                )

        kchunks = [(0, 448), (448, 448), (896, 448), (1344, 224)]
        # pre-beat prefix in x-block readiness order: everything chunk-0's
        # (and h2's) score groups need; q(1..6) and v chains run as fillers
        q_m0(0); k_m0(*kchunks[0]); qk_m1(0); qk_m1(1)
        k_m0(*kchunks[1]); qk_m1(2); qk_m1(3)
        k_m0(*kchunks[2]); k_m0(*kchunks[3])
        qk_m1(4); qk_m1(5); qk_m1(6)
        _inbeat[0] = True

        # ---- B/C: attention, software-pipelined by one q-chunk -----------
        def emit_score_group(h, c, es, grp):
            blk, pb = _head_pos(h)
            if True:
                ng = len(grp)
                scp = SCPS.tile([128, 1024], F32, tag="sc")
                pmax = _ksize(grp[-1])
                for j, kt in enumerate(grp):
                    ksz = _ksize(kt)
                    if kt % 2 == 0:
                        sk, sq, base = kT, qT, pb
                    else:
                        sk, sq, base = kT_dup, qT_dup, HD - pb
                    nc.tensor.matmul(
                        scp[0:ksz, j * 512 : j * 512 + CH],
                        sk[base : base + HD, blk * KP + kt * 128 : blk * KP + kt * 128 + ksz],
                        sq[base : base + HD, blk * Q + c * CH : blk * Q + (c + 1) * CH],
                        start=True,
                        stop=True,
                        tile_position=(base, 0),
                    )
                src = scp[0:pmax, 0 : ng * 512].rearrange(
                    "p (a b) -> p a b", b=512
                )[:, :, 0:CH] if ng > 1 else scp[0:pmax, 0:CH]
                dst = es[
                    0:pmax, grp[0] * CH : (grp[-1] + 1) * CH
                ].rearrange("p (a b) -> p a b", b=CH) if ng > 1 else es[
                    0:pmax, grp[0] * CH : grp[0] * CH + CH
                ]
                nc.scalar.activation(dst, src, AF.Exp, scale=SC)

        def pv_part(h, c, es):
            num = NUMPS.tile([128, 512], F32, tag="num")
            for kt in range(NKT):
                ksz = _ksize(kt)
                slot = (h * NKT + kt) * 65
                nc.tensor.matmul(
                    num[0:65, 0:CH],
                    vv[0:ksz, slot : slot + 65],
                    es[0:ksz, kt * CH : (kt + 1) * CH],
                    start=(kt == 0),
                    stop=(kt == NKT - 1),
                )
            den = INV.tile([128, CH], F32, tag="den")
            nc.vector.tensor_copy(den[0:1, :], num[64:65, 0:CH])
            nsb = INV.tile([128, CH], F32, tag="nsb", bufs=3)
            nc.vector.tensor_copy(nsb[0:64, :], num[0:64, 0:CH])
            inv = INV.tile([128, CH], F32, tag="inv")
            nc.vector.reciprocal_approx_fast(inv[0:1, :], den[0:1, :])
            invr = INV.tile([128, CH], F32R, tag="invr", bufs=3)
            nc.vector.tensor_copy(invr[0:1, :], inv[0:1, :])
            return num, nsb, invr

        def rep_mult(h, c, num, nsb, invr):
            # broadcast 1/den across rows 0:64 of the drained num tile
            # (PV rows already staged into nsb; den row copied out)
            nc.tensor.matmul(
                num[0:64, 0:CH], ones[0:1, 0:64], invr[0:1, 0:CH],
                start=True, stop=True,
            )
            if h == 0:
                dst = attn[0:64, c * CH : (c + 1) * CH]
            elif h == 1:
                dst = attn[64:128, c * CH : (c + 1) * CH]
            else:
                dst = attn[0:64, Q + c * CH : Q + (c + 1) * CH]
            nc.vector.tensor_tensor(
                dst, nsb[0:64, :], num[0:64, 0:CH], op=mybir.AluOpType.mult
            )
            if h == 2:
                # duplicate h2 into rows 64:128 for the concurrent proj pair
                nc.sync.dma_start(
                    attn[64:128, Q + c * CH : Q + (c + 1) * CH],
                    attn[0:64, Q + c * CH : Q + (c + 1) * CH],
                )

        _otst = {}

        def proj_pair(c, mp):
            # out-proj for m-blocks (2*mp, 2*mp+1): two K=128 h0h1 matmuls
            # plus an h2 K=64 pair on opposite PE row halves; bf16 outputs
            # are batched per m-block over chunks (0,1), (2,3), (4,5,6)
            me, mo = 2 * mp, 2 * mp + 1
            pp = SCPS.tile([128, 1024], F32, tag="sc", name=f"pj{mp}")
            nc.tensor.matmul(
                pp[0:128, 0:CH],
                wp_sb[0:128, me * 128 : (me + 1) * 128],
                attn[0:128, c * CH : (c + 1) * CH],
                start=True,
                stop=False,
            )
            nc.tensor.matmul(
                pp[0:128, 512 : 512 + CH],
                wp_sb[0:128, mo * 128 : (mo + 1) * 128],
                attn[0:128, c * CH : (c + 1) * CH],
                start=True,
                stop=False,
            )
            nc.tensor.matmul(
                pp[0:128, 0:CH],
                wph2[0:HD, me * 128 : (me + 1) * 128],
                attn[0:HD, Q + c * CH : Q + (c + 1) * CH],
                start=False,
                stop=True,
                tile_position=(0, 0),
            )
            nc.tensor.matmul(
                pp[0:128, 512 : 512 + CH],
                wph2[HD:128, mo * 128 : (mo + 1) * 128],
                attn[HD:128, Q + c * CH : Q + (c + 1) * CH],
                start=False,
                stop=True,
                tile_position=(HD, 0),
            )
            cb = 2 * (c // 2)                    # output batch start chunk
            half_c = c - cb
            for m, bank in ((me, 0), (mo, 1)):
                if half_c == 0:
                    _otst[m] = OT.tile([128, 2 * CH], BF16, tag=f"otm{m}",
                                       name=f"otm{m}")
                ot = _otst[m]
                nc.vector.tensor_copy(
                    ot[:, half_c * CH : (half_c + 1) * CH],
                    pp[0:128, bank * 512 : bank * 512 + CH],
                )
                if half_c == 1 or c == NCH - 1:
                    w = (half_c + 1) * CH
                    nc.sync.dma_start(
                        out[m * 128 : (m + 1) * 128, cb * CH : cb * CH + w],
                        ot[:, 0:w],
                    )

        pv_queue = []
        drain_state = {}
        for c in range(NCH):
            es = [ES.tile([128, NKT * CH], BF16, tag="es", name=f"es{c}_{h}")
                  for h in range(HPG)]
            fillers = []
            if c == 0:
                fillers = [(lambda cc: lambda: q_m0(cc))(cc)
                           for cc in range(1, NCH)]
                fillers += [(lambda k: lambda: v_kt(k))(kt_m)
                            for kt_m in range(NKT)]
                # interleave: qm0 chains first (they gate later chunks)
                fillers = fillers[:6] + fillers[6:]
            if pv_queue:
                pc = pv_queue[0][1]
                state = {}

                def mk_pv(h, ppc, pes):
                    def f():
                        state[h] = pv_part(h, ppc, pes)
                    return f

                def mk_rep(h, ppc):
                    def f():
                        num, nsb, invr = state[h]
                        rep_mult(h, ppc, num, nsb, invr)
                    return f

                def mk_proj(mp, ppc):
                    def f():
                        proj_pair(ppc, mp)
                    return f

                f_pv = [mk_pv(h, ppc, pes) for h, ppc, pes in pv_queue]
                f_rep = [mk_rep(h, ppc) for h, ppc, pes in pv_queue]
                f_pj = [mk_proj(mp, pc) for mp in range(3)]
                fillers = [
                    f_pv[0], f_pv[1], f_pv[2],
                    f_rep[0], f_rep[1], f_rep[2],
                    f_pj[0], f_pj[1], f_pj[2],
                ]
                if c == NCH - 1:
                    # last chunk: its own h0/h1 PV ride the remaining slack
                    def mk_pv6(h):
                        def f():
                            drain_state[h] = pv_part(h, c, es[h])
                        return f
                    fillers += [mk_pv6(0), mk_pv6(1)]
            # 2-ktile groups each hold one complete row-pair, so fillers
            # can slot between any groups without breaking pairing
            fi = 0
            gi = 0
            for h in range(HPG):
                for grp in EXP_GROUPS:
                    emit_score_group(h, c, es[h], grp)
                    gi += 1
                    if gi % 2 == 0 and fi < len(fillers):
                        fillers[fi]()
                        fi += 1
            while fi < len(fillers):
                fillers[fi]()
                fi += 1
            pv_queue = [(0, c, es[0]), (1, c, es[1]), (2, c, es[2])]
        # drain: h0/h1 PV ran as chunk-6 fillers; divide them first so a
        # num slot frees for h2's PV, then project
        rep_mult(0, NCH - 1, *drain_state[0])
        rep_mult(1, NCH - 1, *drain_state[1])
        st2 = pv_part(2, NCH - 1, pv_queue[2][2])
        rep_mult(2, NCH - 1, *st2)
        for mp in range(3):
            proj_pair(NCH - 1, mp)


def _get_nc():
    if "nc" not in _CACHE:
        _CACHE["nc"] = _build_nc()
    return _CACHE["nc"]


def kernel(x, W_qkv, W_proj, b_proj):
    nc = _get_nc()
    xTs = [
        np.ascontiguousarray(
            x[n].reshape(Q, D).astype(BFNP).T
        )
        for n in range(N)
    ]
    wqs, wks, wvs, wps = [], [], [], []
    for g in range(4):
        c0 = g * GD
        wqs.append(np.ascontiguousarray(W_qkv[:, c0 : c0 + GD].astype(BFNP)))
        wks.append(np.ascontiguousarray(W_qkv[:, D + c0 : D + c0 + GD].astype(BFNP)))
        wvs.append(
            np.ascontiguousarray(W_qkv[:, 2 * D + c0 : 2 * D + c0 + GD].astype(BFNP))
        )
        wps.append(np.ascontiguousarray(W_proj[c0 : c0 + GD, :].astype(BFNP)))
    in_maps = [
        {"xT": xTs[c // 4], "wq": wqs[c % 4], "wk": wks[c % 4],
         "wv": wvs[c % 4], "wp": wps[c % 4]}
        for c in range(N_CORES)
    ]
    res = run_bass_kernel_spmd(nc, in_maps, list(range(N_CORES)), trace=TRACE)
    if TRACE:
        LAST_RESULTS["exec_time_ns"] = res.exec_time_ns
        LAST_RESULTS["mean_exec_time_ns"] = res.mean_exec_time_ns
    out = np.empty((N, T, S, D), np.float32)
    for n in range(N):
        acc = res.results[4 * n]["out"].astype(np.float32)
        for g in range(1, 4):
            acc = acc + res.results[4 * n + g]["out"].astype(np.float32)
        out[n] = (acc.T + b_proj).reshape(T, S, D)
    return out

